# revision 1
# baseline (speedup 1.0000x reference)
"""Trainium2 Bass kernel for nn_Aggregator (GNN message passing).

Reference computation (fp32):
    neigh_agg = mean(x_neigh, axis=2) @ w_neigh     # (B,H,25,128) -> (B,H,128)
    self_agg  = x_self @ w_self                     # (B,H,128)   -> (B,H,128)
    out = relu(concat([self_agg, neigh_agg], -1) + bias)   # (B,H,256)

Strategy: data-parallel over the B axis across 8 NeuronCores. Per core,
rows = (B/8)*H = 10240 rows, processed in 80 blocks of 128 rows:
  - DMA x_neigh block [128, 25*128] (12.8KB contiguous per partition line)
  - DVE tree-reduction over the 25 neighbor chunks (5 in-place adds)
  - PE transpose (via identity matmul) of the reduced block and the x_self
    block to get features on partitions
  - PE matmuls: bias seed (K=1, ones x bias row), self & neigh projections
    accumulated into one PSUM tile [128, 256]
  - ACT relu PSUM -> SBUF, DMA out
The 1/25 mean factor is folded into w_neigh host-side. All constants are
packed into ONE DRAM tensor / one DMA so PE instructions never need more
than one semaphore wait (walrus limit on self-loading fp32 Matmult).
"""

import sys

for _p in ("/opt/trn_rl_repo", "/root/.axon_site/_ro/trn_rl_repo"):
    if _p not in sys.path:
        sys.path.append(_p)

import numpy as np

from concourse import bacc, bass, mybir
from concourse.bass_utils import run_bass_kernel_spmd
from concourse.tile import TileContext

N_CORES = 8
B, H, NN, F = 8192, 10, 25, 128
D = 256
B_LOC = B // N_CORES          # 1024
R_LOC = B_LOC * H             # 10240 rows per core
P = 128                       # partition block
N_BLOCKS = R_LOC // P         # 80
FP32 = mybir.dt.float32
RELU = mybir.ActivationFunctionType.Relu

# Packed constant layout (columns of a [128, CW] tensor):
#   [0:128)   w_self
#   [128:256) w_neigh / 25
#   [256:384) identity
#   row 0 [384:640) bias, [640:768) ones
CW = 768


def build_bass(loop_iters=None, bpt=1, xn_bufs=None, unroll_reps=1):
    """bpt = 128-row blocks per x_neigh SBUF tile (1, 2 or 4). For bpt >= 2
    the tile holds bpt sub-blocks along the free dim and the load is issued
    as two large DMAs (one per HWDGE ring), each covering bpt/2 sub-blocks
    of contiguous DRAM. For bpt == 1 the single block's columns are split
    across the two rings."""
    assert bpt in (1, 2, 4) and N_BLOCKS % bpt == 0
    if xn_bufs is None:
        xn_bufs = {1: 6, 2: 3, 4: 2}[bpt]
    CF = NN * F  # 3200 columns per 128-row sub-block

    nc = bacc.Bacc(None)
    xs = nc.dram_tensor("xs", [R_LOC, F], FP32, kind="ExternalInput")
    xn = nc.dram_tensor("xn", [R_LOC, CF], FP32, kind="ExternalInput")
    consts = nc.dram_tensor("consts", [P, CW], FP32, kind="ExternalInput")
    out = nc.dram_tensor("out", [R_LOC, D], FP32, kind="ExternalOutput")

    with TileContext(nc) as tc:
        if loop_iters is not None:
            loop_cm = tc.For_i(0, loop_iters, 1)
            loop_cm.__enter__()
        with (
            tc.tile_pool(name="const", bufs=1) as cpool,
            tc.tile_pool(name="xn", bufs=xn_bufs) as xnpool,
            tc.tile_pool(name="xs", bufs=4) as xspool,
            tc.tile_pool(name="tsb", bufs=3) as tpool,
            tc.tile_pool(name="osb", bufs=3) as opool,
            tc.tile_pool(name="pst", bufs=2, space="PSUM") as pspool_t,
            tc.tile_pool(name="pso", bufs=4, space="PSUM") as pspool_o,
        ):
            const_t = cpool.tile([P, CW], FP32)
            nc.sync.dma_start(out=const_t, in_=consts[:, :])
            wself_ap = const_t[:, 0:F]
            wneigh_ap = const_t[:, F : 2 * F]
            ident_ap = const_t[:, 2 * F : 3 * F]
            bias_ap = const_t[0:1, 3 * F : 3 * F + D]
            ones_ap = const_t[0:1, 3 * F + D : 3 * F + D + P]

            def emit_block(r0, xn_view, eng):
                """Process one 128-row block whose x_neigh data (25 chunks of
                F) sits in SBUF at xn_view. eng issues the small DMAs."""
                r1 = r0 + P

                # Seed PSUM rows with the bias: out[j, d] = ones[0,j]*bias[0,d].
                # Emitted first so the PE's vector clock covers the const DMA
                # before any other PE instruction (1-wait limit on Matmult).
                o_ps = pspool_o.tile([P, D], FP32)
                nc.tensor.matmul(
                    out=o_ps[:, :], lhsT=ones_ap, rhs=bias_ap,
                    start=True, stop=False, skip_group_check=True,
                )

                # Tree-reduce 25 chunks of width F down to xn_view[:, 0:F].
                nc.vector.tensor_add(
                    out=xn_view[:, 0 : 9 * F],
                    in0=xn_view[:, 0 : 9 * F],
                    in1=xn_view[:, 16 * F : 25 * F],
                )
                nc.vector.tensor_add(
                    out=xn_view[:, 0 : 8 * F],
                    in0=xn_view[:, 0 : 8 * F],
                    in1=xn_view[:, 8 * F : 16 * F],
                )
                nc.vector.tensor_add(
                    out=xn_view[:, 0 : 4 * F],
                    in0=xn_view[:, 0 : 4 * F],
                    in1=xn_view[:, 4 * F : 8 * F],
                )
                nc.vector.tensor_add(
                    out=xn_view[:, 0 : 2 * F],
                    in0=xn_view[:, 0 : 2 * F],
                    in1=xn_view[:, 2 * F : 4 * F],
                )
                nc.vector.tensor_add(
                    out=xn_view[:, 0:F],
                    in0=xn_view[:, 0:F],
                    in1=xn_view[:, F : 2 * F],
                )

                # Transpose reduced neigh block: [rows, f] -> [f, rows]
                sT_ps = pspool_t.tile([P, P], FP32)
                nc.tensor.transpose(out=sT_ps, in_=xn_view[:, 0:F], identity=ident_ap)
                sT = tpool.tile([P, P], FP32)
                nc.scalar.copy(out=sT, in_=sT_ps)

                xs_t = xspool.tile([P, F], FP32)
                eng.dma_start(out=xs_t, in_=xs[r0:r1, :])
                xsT_ps = pspool_t.tile([P, P], FP32)
                nc.tensor.transpose(out=xsT_ps, in_=xs_t, identity=ident_ap)
                xsT = tpool.tile([P, P], FP32)
                nc.scalar.copy(out=xsT, in_=xsT_ps)

                nc.tensor.matmul(
                    out=o_ps[:, 0:F], lhsT=xsT, rhs=wself_ap,
                    start=False, stop=False, skip_group_check=True,
                )
                nc.tensor.matmul(
                    out=o_ps[:, F:D], lhsT=sT, rhs=wneigh_ap,
                    start=False, stop=True, skip_group_check=True,
                )

                o_sb = opool.tile([P, D], FP32)
                nc.scalar.activation(out=o_sb, in_=o_ps, func=RELU)
                eng.dma_start(out=out[r0:r1, :], in_=o_sb)

            for _rep in range(unroll_reps):
                if bpt == 1:
                    for i in range(N_BLOCKS):
                        r0 = i * P
                        xn_t = xnpool.tile([P, CF], FP32)
                        # Split the 1.6MB block load across both HWDGE rings.
                        nc.sync.dma_start(
                            out=xn_t[:, 0 : 16 * F], in_=xn[r0 : r0 + P, 0 : 16 * F]
                        )
                        nc.scalar.dma_start(
                            out=xn_t[:, 16 * F :], in_=xn[r0 : r0 + P, 16 * F :]
                        )
                        emit_block(r0, xn_t, nc.sync if i % 2 == 0 else nc.scalar)
                else:
                    half = bpt // 2
                    for s in range(N_BLOCKS // bpt):
                        r0 = s * bpt * P
                        xn_t = xnpool.tile([P, bpt * CF], FP32)
                        # Each ring loads bpt/2 sub-blocks (contiguous DRAM
                        # rows) as one large DMA.
                        for h, eng in ((0, nc.sync), (1, nc.scalar)):
                            rows0 = r0 + h * half * P
                            dst = xn_t[:, h * half * CF : (h + 1) * half * CF]
                            if half == 1:
                                eng.dma_start(out=dst, in_=xn[rows0 : rows0 + P, :])
                            else:
                                eng.dma_start(
                                    out=dst.rearrange("p (j f) -> p j f", j=half),
                                    in_=xn[rows0 : rows0 + half * P, :].rearrange(
                                        "(j p) f -> p j f", j=half
                                    ),
                                )
                        for j in range(bpt):
                            emit_block(
                                r0 + j * P,
                                xn_t[:, j * CF : (j + 1) * CF],
                                nc.sync if j % 2 == 0 else nc.scalar,
                            )

        if loop_iters is not None:
            loop_cm.__exit__(None, None, None)

    nc.compile()
    return nc


_NC_CACHE = None


def kernel(x_self, x_neigh, w_neigh, w_self, bias):
    global _NC_CACHE
    if _NC_CACHE is None:
        _NC_CACHE = build_bass()
    nc = _NC_CACHE

    x_self = np.ascontiguousarray(x_self, dtype=np.float32)
    x_neigh = np.ascontiguousarray(x_neigh, dtype=np.float32)

    consts = np.zeros((P, CW), dtype=np.float32)
    consts[:, 0:F] = np.asarray(w_self, dtype=np.float32)
    consts[:, F : 2 * F] = np.asarray(w_neigh, dtype=np.float32) / np.float32(NN)
    consts[:, 2 * F : 3 * F] = np.eye(P, dtype=np.float32)
    consts[0, 3 * F : 3 * F + D] = np.asarray(bias, dtype=np.float32)
    consts[0, 3 * F + D : 3 * F + D + P] = 1.0

    in_maps = []
    for c in range(N_CORES):
        b0, b1 = c * B_LOC, (c + 1) * B_LOC
        in_maps.append(
            {
                "xs": x_self[b0:b1].reshape(R_LOC, F),
                "xn": x_neigh[b0:b1].reshape(R_LOC, NN * F),
                "consts": consts,
            }
        )

    res = run_bass_kernel_spmd(nc, in_maps, list(range(N_CORES)))
    out = np.concatenate([res.results[c]["out"] for c in range(N_CORES)], axis=0)
    return out.reshape(B, H, D)



# revision 2
# speedup vs baseline: 1.5800x; 1.5800x over previous
"""Trainium2 Bass kernel for nn_Aggregator (GNN message passing).

v4 (weights-stationary, fp8 moving xn, G=4 blocks/group) left the PE as
the bottleneck (~300ns per N=512 matmul on dense data). v5 splits the
neighbor reduction across engines:
  - PE: chunks 0..12 as accumulating matmuls (ring A data), plus the
    DVE-reduced pseudo-chunk, plus the self matmul.
  - DVE: tree-reduces chunks 13..24 (ring B data) in bf16 (11 adds,
    fp8 inputs upcast on the first level) into one [f, GP] chunk.
ACT fuses per-partition bias + relu + bf16 downcast; stores ride the
gpsimd SWDGE queue so the HWDGE rings only ever carry loads.

Numerics: fp8-e4m3 xn with bf16 tree + fp32 PSUM -> rel-to-max ~5e-3
(gate 2e-2). Traffic/core: 32.8MB xn + 2.6MB xs + 5.2MB out = 40.6MB.
"""

import sys

for _p in ("/opt/trn_rl_repo", "/root/.axon_site/_ro/trn_rl_repo"):
    if _p not in sys.path:
        sys.path.append(_p)

import numpy as np

from concourse import bacc, bass, mybir
from concourse.bass_utils import run_bass_kernel_spmd
from concourse.tile import TileContext

N_CORES = 8
B, H, NN, F = 8192, 10, 25, 128
D = 256
B_LOC = B // N_CORES          # 1024
R_LOC = B_LOC * H             # 10240 rows per core
P = 128
N_BLOCKS = R_LOC // P         # 80
G = 4                         # row-blocks per group
GP = G * P                    # 512 moving columns
N_GROUPS = N_BLOCKS // G      # 20
FP32 = mybir.dt.float32
BF16 = mybir.dt.bfloat16
FP8 = mybir.dt.float8e4
RELU = mybir.ActivationFunctionType.Relu

CN = NN * GP                  # 12800 xnt columns per group row
CW = 258                      # consts: ws, wn/25, bias_self col, bias_neigh col

PE_CHUNKS = 13                # chunks 0..12 on the PE (ring A)
DVE_CHUNKS = NN - PE_CHUNKS   # chunks 13..24 on the DVE (ring B)


def build_bass(loop_iters=None, bpt=1, xn_bufs=None, unroll_reps=1,
               xn_dtype=FP8, pe_chunks=PE_CHUNKS):
    if xn_bufs is None:
        xn_bufs = 5
    RS = pe_chunks
    nd = NN - RS              # chunks reduced on DVE

    nc = bacc.Bacc(None)
    xst = nc.dram_tensor("xst", [N_GROUPS * F, GP], BF16, kind="ExternalInput")
    xnt = nc.dram_tensor("xnt", [N_GROUPS * F, CN], xn_dtype, kind="ExternalInput")
    consts = nc.dram_tensor("consts", [P, CW], BF16, kind="ExternalInput")
    out = nc.dram_tensor("out", [N_GROUPS * P, 2 * GP], BF16, kind="ExternalOutput")

    with TileContext(nc) as tc:
        if loop_iters is not None:
            loop_cm = tc.For_i(0, loop_iters, 1)
            loop_cm.__enter__()
        with (
            tc.tile_pool(name="const", bufs=1) as cpool,
            tc.tile_pool(name="xn", bufs=xn_bufs) as xnpool,
            tc.tile_pool(name="xs", bufs=4) as xspool,
            tc.tile_pool(name="red", bufs=3) as rpool,
            tc.tile_pool(name="osb", bufs=3) as opool,
            tc.tile_pool(name="psS", bufs=2, space="PSUM") as pspool_s,
            tc.tile_pool(name="psN", bufs=3, space="PSUM") as pspool_n,
        ):
            const_t = cpool.tile([P, CW], BF16)
            nc.sync.dma_start(out=const_t, in_=consts[:, :])
            ws_ap = const_t[:, 0:F]
            wn_ap = const_t[:, F : 2 * F]
            bias_s_ap = const_t[:, 2 * F : 2 * F + 1]
            bias_n_ap = const_t[:, 2 * F + 1 : 2 * F + 2]

            for _rep in range(unroll_reps):
                for g in range(N_GROUPS):
                    f0 = g * F
                    xn_t = xnpool.tile([P, CN], xn_dtype)
                    # Ring A: PE-direct chunks. Ring B: xs, then DVE chunks.
                    nc.sync.dma_start(
                        out=xn_t[:, 0 : RS * GP], in_=xnt[f0 : f0 + F, 0 : RS * GP]
                    )
                    xs_t = xspool.tile([P, GP], BF16)
                    nc.scalar.dma_start(out=xs_t, in_=xst[f0 : f0 + F, :])
                    nc.scalar.dma_start(
                        out=xn_t[:, RS * GP :], in_=xnt[f0 : f0 + F, RS * GP :]
                    )

                    # DVE pairwise tree over chunks RS..NN-1 -> red[:, 0:GP].
                    # Level 1 upcasts fp8 -> bf16; the rest stays bf16.
                    npairs = nd // 2
                    red = rpool.tile([P, npairs * GP], BF16)

                    def ck(n):
                        return xn_t[:, n * GP : (n + 1) * GP]

                    def rd(k):
                        return red[:, k * GP : (k + 1) * GP]

                    for k in range(npairs):
                        nc.vector.tensor_add(
                            out=rd(k), in0=ck(RS + 2 * k), in1=ck(RS + 2 * k + 1)
                        )
                    # nd may be odd: fold the leftover chunk into the last slot
                    # via one more (mixed-width) add after level 1.
                    if nd % 2 == 1:
                        nc.vector.tensor_add(
                            out=rd(npairs - 1), in0=rd(npairs - 1), in1=ck(NN - 1)
                        )
                    w = npairs
                    while w > 1:
                        h = w // 2
                        for k in range(h):
                            nc.vector.tensor_add(
                                out=rd(k), in0=rd(k), in1=rd(k + h)
                            )
                        if w % 2 == 1:
                            nc.vector.tensor_add(
                                out=rd(h - 1), in0=rd(h - 1), in1=rd(w - 1)
                            )
                        w = h

                    self_ps = pspool_s.tile([P, GP], FP32)
                    neigh_ps = pspool_n.tile([P, GP], FP32)

                    for n in range(RS):
                        nc.tensor.matmul(
                            out=neigh_ps,
                            lhsT=wn_ap,
                            rhs=ck(n),
                            start=(n == 0), stop=False, skip_group_check=True,
                        )
                    nc.tensor.matmul(
                        out=neigh_ps, lhsT=wn_ap, rhs=rd(0),
                        start=False, stop=True, skip_group_check=True,
                    )
                    nc.tensor.matmul(
                        out=self_ps, lhsT=ws_ap, rhs=xs_t,
                        start=True, stop=True, skip_group_check=True,
                    )

                    o_sb = opool.tile([P, 2 * GP], BF16)
                    nc.scalar.activation(
                        out=o_sb[:, 0:GP], in_=self_ps, func=RELU, bias=bias_s_ap
                    )
                    nc.scalar.activation(
                        out=o_sb[:, GP : 2 * GP], in_=neigh_ps, func=RELU,
                        bias=bias_n_ap,
                    )
                    # Stores alone on the gpsimd SWDGE queue.
                    nc.gpsimd.dma_start(
                        out=out[g * P : (g + 1) * P, :], in_=o_sb
                    )

        if loop_iters is not None:
            loop_cm.__exit__(None, None, None)

    nc.compile()
    return nc


_NC_CACHE = None


def kernel(x_self, x_neigh, w_neigh, w_self, bias):
    import ml_dtypes

    global _NC_CACHE
    if _NC_CACHE is None:
        _NC_CACHE = build_bass()
    nc = _NC_CACHE

    ng = N_CORES * N_GROUPS
    xn8 = np.asarray(x_neigh).astype(ml_dtypes.float8_e4m3)
    xn8 = xn8.reshape(ng, G, P, NN, F)
    xnt = np.ascontiguousarray(xn8.transpose(0, 4, 3, 1, 2)).reshape(ng * F, CN)
    xsb = np.asarray(x_self).astype(ml_dtypes.bfloat16).reshape(ng, G, P, F)
    xst = np.ascontiguousarray(xsb.transpose(0, 3, 1, 2)).reshape(ng * F, GP)

    consts = np.zeros((P, CW), dtype=np.float32)
    consts[:, 0:F] = np.asarray(w_self, dtype=np.float32)
    consts[:, F : 2 * F] = np.asarray(w_neigh, dtype=np.float32) / np.float32(NN)
    consts[:, 2 * F] = np.asarray(bias, dtype=np.float32)[0:P]
    consts[:, 2 * F + 1] = np.asarray(bias, dtype=np.float32)[P:D]
    consts = consts.astype(ml_dtypes.bfloat16)

    rg = N_GROUPS * F
    in_maps = [
        {"xst": xst[c * rg : (c + 1) * rg], "xnt": xnt[c * rg : (c + 1) * rg],
         "consts": consts}
        for c in range(N_CORES)
    ]

    res = run_bass_kernel_spmd(nc, in_maps, list(range(N_CORES)))
    out = np.concatenate([res.results[c]["out"] for c in range(N_CORES)], axis=0)
    # out[g*P + d, h*GP + j*P + r] -> full[(g*G+j)*P + r, h*P + d]
    o = out.astype(np.float32).reshape(ng, P, 2, G, P)
    o = o.transpose(0, 3, 4, 2, 1).reshape(B, H, D)
    return o


# revision 5
# speedup vs baseline: 1.6310x; 1.0323x over previous
"""Trainium2 Bass kernel for nn_Aggregator (GNN message passing).

v4 (weights-stationary, fp8 moving xn, G=4 blocks/group) left the PE as
the bottleneck (~300ns per N=512 matmul on dense data). v5 splits the
neighbor reduction across engines:
  - PE: chunks 0..12 as accumulating matmuls (ring A data), plus the
    DVE-reduced pseudo-chunk, plus the self matmul.
  - DVE: tree-reduces chunks 13..24 (ring B data) in bf16 (11 adds,
    fp8 inputs upcast on the first level) into one [f, GP] chunk.
ACT fuses per-partition bias + relu + bf16 downcast; stores ride the
gpsimd SWDGE queue so the HWDGE rings only ever carry loads.

Numerics: fp8-e4m3 xn with bf16 tree + fp32 PSUM -> rel-to-max ~5e-3
(gate 2e-2). Traffic/core: 32.8MB xn + 2.6MB xs + 5.2MB out = 40.6MB.
"""

import sys

for _p in ("/opt/trn_rl_repo", "/root/.axon_site/_ro/trn_rl_repo"):
    if _p not in sys.path:
        sys.path.append(_p)

import numpy as np

from concourse import bacc, bass, mybir
from concourse.bass_utils import run_bass_kernel_spmd
from concourse.tile import TileContext

N_CORES = 8
B, H, NN, F = 8192, 10, 25, 128
D = 256
B_LOC = B // N_CORES          # 1024
R_LOC = B_LOC * H             # 10240 rows per core
P = 128
N_BLOCKS = R_LOC // P         # 80
G = 4                         # row-blocks per group
GP = G * P                    # 512 moving columns
N_GROUPS = N_BLOCKS // G      # 20
FP32 = mybir.dt.float32
BF16 = mybir.dt.bfloat16
FP8 = mybir.dt.float8e4
RELU = mybir.ActivationFunctionType.Relu

CN = NN * GP                  # 12800 xnt columns per group row
CW = 258                      # consts: ws, wn/25, bias_self col, bias_neigh col

PE_CHUNKS = 13                # chunks 0..12 on the PE (ring A)
DVE_CHUNKS = NN - PE_CHUNKS   # chunks 13..24 on the DVE (ring B)


def build_bass(loop_iters=None, bpt=1, xn_bufs=None, unroll_reps=1,
               xn_dtype=FP8, pe_chunks=PE_CHUNKS, psn_bufs=3, osb_bufs=3):
    if xn_bufs is None:
        xn_bufs = 5
    RS = pe_chunks
    nd = NN - RS              # chunks reduced on DVE

    nc = bacc.Bacc(None)
    xst = nc.dram_tensor("xst", [N_GROUPS * F, GP], BF16, kind="ExternalInput")
    xnt = nc.dram_tensor("xnt", [N_GROUPS * F, CN], xn_dtype, kind="ExternalInput")
    consts = nc.dram_tensor("consts", [P, CW], BF16, kind="ExternalInput")
    out = nc.dram_tensor("out", [N_GROUPS * P, 2 * GP], BF16, kind="ExternalOutput")

    with TileContext(nc) as tc:
        if loop_iters is not None:
            loop_cm = tc.For_i(0, loop_iters, 1)
            loop_cm.__enter__()
        with (
            tc.tile_pool(name="const", bufs=1) as cpool,
            tc.tile_pool(name="xn", bufs=xn_bufs) as xnpool,
            tc.tile_pool(name="xs", bufs=4) as xspool,
            tc.tile_pool(name="red", bufs=3) as rpool,
            tc.tile_pool(name="osb", bufs=osb_bufs) as opool,
            tc.tile_pool(name="psS", bufs=2, space="PSUM") as pspool_s,
            tc.tile_pool(name="psN", bufs=psn_bufs, space="PSUM") as pspool_n,
        ):
            const_t = cpool.tile([P, CW], BF16)
            nc.sync.dma_start(out=const_t, in_=consts[:, :])
            ws_ap = const_t[:, 0:F]
            wn_ap = const_t[:, F : 2 * F]
            bias_s_ap = const_t[:, 2 * F : 2 * F + 1]
            bias_n_ap = const_t[:, 2 * F + 1 : 2 * F + 2]

            for _rep in range(unroll_reps):
                for g in range(N_GROUPS):
                    f0 = g * F
                    xn_t = xnpool.tile([P, CN], xn_dtype)
                    # Ring A: PE-direct chunks. Ring B: xs, then DVE chunks.
                    nc.sync.dma_start(
                        out=xn_t[:, 0 : RS * GP], in_=xnt[f0 : f0 + F, 0 : RS * GP]
                    )
                    xs_t = xspool.tile([P, GP], BF16)
                    nc.scalar.dma_start(out=xs_t, in_=xst[f0 : f0 + F, :])
                    nc.scalar.dma_start(
                        out=xn_t[:, RS * GP :], in_=xnt[f0 : f0 + F, RS * GP :]
                    )

                    # DVE reduction of chunks RS..NN-1 -> red[:, 0:GP] in 4
                    # wide strip ops (pairing is arbitrary for a sum, so wide
                    # contiguous strips replace narrow per-pair adds — same
                    # element work, far less per-instruction overhead).
                    # Level 1 adds the two strip halves (fp8 -> bf16), then
                    # contiguous halving on the bf16 partials.
                    assert nd % 2 == 0
                    npar = nd // 2
                    red = rpool.tile([P, npar * GP], BF16)

                    def ck(n):
                        return xn_t[:, n * GP : (n + 1) * GP]

                    nc.vector.tensor_add(
                        out=red,
                        in0=xn_t[:, RS * GP : (RS + npar) * GP],
                        in1=xn_t[:, (RS + npar) * GP : NN * GP],
                    )
                    w = npar
                    while w > 1:
                        h = w // 2
                        nc.vector.tensor_add(
                            out=red[:, 0 : h * GP],
                            in0=red[:, 0 : h * GP],
                            in1=red[:, (w - h) * GP : w * GP],
                        )
                        w -= h

                    self_ps = pspool_s.tile([P, GP], FP32)
                    neigh_ps = pspool_n.tile([P, GP], FP32)

                    for n in range(RS):
                        nc.tensor.matmul(
                            out=neigh_ps,
                            lhsT=wn_ap,
                            rhs=ck(n),
                            start=(n == 0), stop=False, skip_group_check=True,
                        )
                    nc.tensor.matmul(
                        out=neigh_ps, lhsT=wn_ap, rhs=red[:, 0:GP],
                        start=False, stop=True, skip_group_check=True,
                    )
                    nc.tensor.matmul(
                        out=self_ps, lhsT=ws_ap, rhs=xs_t,
                        start=True, stop=True, skip_group_check=True,
                    )

                    o_sb = opool.tile([P, 2 * GP], BF16)
                    nc.scalar.activation(
                        out=o_sb[:, 0:GP], in_=self_ps, func=RELU, bias=bias_s_ap
                    )
                    nc.scalar.activation(
                        out=o_sb[:, GP : 2 * GP], in_=neigh_ps, func=RELU,
                        bias=bias_n_ap,
                    )
                    # Stores alone on the gpsimd SWDGE queue.
                    nc.gpsimd.dma_start(
                        out=out[g * P : (g + 1) * P, :], in_=o_sb
                    )

        if loop_iters is not None:
            loop_cm.__exit__(None, None, None)

    nc.compile()
    return nc


_NC_CACHE = None


def kernel(x_self, x_neigh, w_neigh, w_self, bias):
    import ml_dtypes

    global _NC_CACHE
    if _NC_CACHE is None:
        _NC_CACHE = build_bass()
    nc = _NC_CACHE

    ng = N_CORES * N_GROUPS
    xn8 = np.asarray(x_neigh).astype(ml_dtypes.float8_e4m3)
    xn8 = xn8.reshape(ng, G, P, NN, F)
    xnt = np.ascontiguousarray(xn8.transpose(0, 4, 3, 1, 2)).reshape(ng * F, CN)
    xsb = np.asarray(x_self).astype(ml_dtypes.bfloat16).reshape(ng, G, P, F)
    xst = np.ascontiguousarray(xsb.transpose(0, 3, 1, 2)).reshape(ng * F, GP)

    consts = np.zeros((P, CW), dtype=np.float32)
    consts[:, 0:F] = np.asarray(w_self, dtype=np.float32)
    consts[:, F : 2 * F] = np.asarray(w_neigh, dtype=np.float32) / np.float32(NN)
    consts[:, 2 * F] = np.asarray(bias, dtype=np.float32)[0:P]
    consts[:, 2 * F + 1] = np.asarray(bias, dtype=np.float32)[P:D]
    consts = consts.astype(ml_dtypes.bfloat16)

    rg = N_GROUPS * F
    in_maps = [
        {"xst": xst[c * rg : (c + 1) * rg], "xnt": xnt[c * rg : (c + 1) * rg],
         "consts": consts}
        for c in range(N_CORES)
    ]

    res = run_bass_kernel_spmd(nc, in_maps, list(range(N_CORES)))
    out = np.concatenate([res.results[c]["out"] for c in range(N_CORES)], axis=0)
    # out[g*P + d, h*GP + j*P + r] -> full[(g*G+j)*P + r, h*P + d]
    o = out.astype(np.float32).reshape(ng, P, 2, G, P)
    o = o.transpose(0, 3, 4, 2, 1).reshape(B, H, D)
    return o
